# revision 1
# baseline (speedup 1.0000x reference)
"""SE(3) diffusion scheduler add-noise kernel for 8 Trainium2 NeuronCores.

Math: reference computes
    orig = se3_exp(twist); xi = se3_log(inv(orig));
    H_t = se3_exp((1-sqrt(ab))*xi) @ orig;  H_n = se3_exp(sqrt(1-ab)*scale*noise)
    out0 = H_n @ H_t; out1 = H_n
Since exp(a*xi)exp(b*xi) = exp((a+b)*xi) on the one-parameter subgroup and
rotation angles stay < pi here (twist = 0.5*randn), xi = -twist exactly and
    H_t = se3_exp(sqrt(ab) * twist).
Validated against float64: the reference deviates from this closed form only
by its own f32 roundtrip noise (fro rel ~7e-7).

Layout: pure data-parallel over B. Per core 512*64 = 32768 samples as
[128 partitions x 256 free] planes. Rotations via half-angle quaternions,
compose via quaternion product; translations via t = a*v + b*(w x v)
+ c*(w x (w x v)).

Perf notes: DVE runs 2-byte-dtype tensor_tensor at 2 elem/cycle/lane
(2x_1p) and tensor_copy at 2x for any dtype/stride (2x_2p), while f32
tensor_tensor and all scalar_tensor_tensor run at 1x. So the bulk compute
is fp16 with plain TT ops (pre-scaling via ACT's free affine instead of
STT), the angle chain (sum-squares -> sqrt -> reciprocal) stays f32, and
results land in fp16 staging tiles (plane index = output entry j) that are
scattered into the sample-interleaved f32 output tiles with one strided
2x copy each. ACT ordering keeps both Sqrt ops ahead of every Sin so the
activation table set switches once.
"""

import os
import sys

import numpy as np

for _p in ("/opt/trn_rl_repo", "/root/.axon_site/_ro/trn_rl_repo"):
    if os.path.isdir(_p) and _p not in sys.path:
        sys.path.append(_p)

N_CORES = 8
B, HO = 4096, 64
BL = B // N_CORES           # 512 rows per core
NS = BL * HO                # 32768 samples per core
P, F = 128, 256             # plane geometry: NS = P*F
PI_HALF = 1.5707963267948966
SQ2 = 1.4142135623730951

_CACHE: dict = {}


def _build_program():
    import concourse.bacc as bacc
    import concourse.mybir as mybir
    import concourse.tile as tile
    from concourse.bass import AP

    f32 = mybir.dt.float32
    f16 = mybir.dt.float16
    Sin = mybir.ActivationFunctionType.Sin
    Sqrt = mybir.ActivationFunctionType.Sqrt
    Square = mybir.ActivationFunctionType.Square
    Copy = mybir.ActivationFunctionType.Copy
    ADD = mybir.AluOpType.add

    nc = bacc.Bacc("TRN2", target_bir_lowering=False, debug=False, num_devices=1)

    tw_d = nc.dram_tensor("tw", [P, 6 * F], f16, kind="ExternalInput").ap()
    ns_d = nc.dram_tensor("ns", [P, 6 * F], f16, kind="ExternalInput").ap()
    sq_d = nc.dram_tensor("sq", [P, 3 * F], f16, kind="ExternalInput").ap()
    o0_d = nc.dram_tensor("o0", [P, 16 * F], f32, kind="ExternalOutput").ap()
    o1_d = nc.dram_tensor("o1", [P, 16 * F], f32, kind="ExternalOutput").ap()

    def bc3(plane):
        """[P,F] plane AP -> broadcast [P,3,F] AP (stride-0 middle dim)."""
        a = plane
        return AP(a.tensor, a.offset, [list(a.ap[0]), [0, 3], list(a.ap[-1])])

    def c3(t):
        return t[:].rearrange("p (c f) -> p c f", c=3)

    def tri(t, p0, dp):
        """[P,*] tile -> [P,3,F] AP of planes p0, p0+dp, p0+2dp."""
        a = t[:, p0 * F:(p0 + 1) * F]
        return AP(a.tensor, a.offset, [list(a.ap[0]), [dp * F, 3], list(a.ap[-1])])

    n_reps = int(os.environ.get("KERNEL_REPS", "1"))

    with tile.TileContext(nc) as tc:
        with tc.tile_pool(name="w", bufs=1) as pool:
            V, A, G = nc.vector, nc.scalar, nc.gpsimd

            def T(cols, tag, dt=f16):
                return pool.tile([P, cols], dt, tag=tag, name=tag)

            def pl(t, k):
                return t[:, k * F:(k + 1) * F]

            for _rep in range(n_reps):
                # ---- inputs (fp16, already component-planes from host) ----
                tw6 = T(6 * F, "tw6"); ns6 = T(6 * F, "ns6")
                sqh = T(3 * F, "sqh")
                nc.sync.dma_start(sqh[:], sq_d[:])
                nc.sync.dma_start(ns6[:], ns_d[:])
                nc.sync.dma_start(tw6[:], tw_d[:])
                # plane order [qr | s | qt]: SD = dual rotation-scale [qr|s]
                SD = sqh[:, 0:2 * F]
                S16 = sqh[:, F:2 * F]
                QT16 = pl(sqh, 2)

                # ---- outputs (f32, interleaved: sample f at cols f*16+j) ----
                o0 = T(16 * F, "o0", f32); o1 = T(16 * F, "o1", f32)
                o0v = o0[:].rearrange("p (f j) -> p f j", j=16)
                o1v = o1[:].rearrange("p (f j) -> p f j", j=16)

                pih = T(1, "pih", f32)                 # pi/2 bias for cos-via-sin
                G.memset(pih[:], PI_HALF)
                # prefetch the sqrt act-table set while input DMAs run
                dummy = T(1, "dummy", f32)
                A.activation(dummy[:], pih[:], Sqrt)

                # fp16 staging tiles: plane index = output entry j (0..11)
                stO = T(12 * F, "stO"); stN = T(12 * F, "stN")

                # ======== phase 1 (f32): th2 per chain -> dual [P,2F] tile ====
                F2 = 2 * F
                th2d = T(F2, "th2d", f32)

                def chain_pre(pre, w6_h, half):
                    sq = T(3 * F, pre + "sq", f32)
                    A.activation(sq[:], w6_h[:, 0:3 * F], Square)
                    ta = T(F, pre + "ta", f32)
                    V.tensor_add(ta[:], pl(sq, 0), pl(sq, 1))
                    V.scalar_tensor_tensor(th2d[:, half * F:(half + 1) * F],
                                           ta[:], 1e-30, pl(sq, 2),
                                           op0=ADD, op1=ADD)

                chain_pre("N", ns6, 0)
                chain_pre("T", tw6, 1)
                thd = T(F2, "thd")             # fp16 sqrt straight off ACT
                A.activation(thd[:], th2d[:], Sqrt)
                rh2f = T(F2, "rh2f", f32)
                V.reciprocal_approx_fast(rh2f[:], th2d[:])
                rh2d = T(F2, "rh2d")
                V.tensor_copy(rh2d[:], rh2f[:])
                # prefetch the trig act-table set right after the real Sqrt
                # (reading thd so the scheduler cannot hoist it earlier)
                dummy2 = T(1, "dummy2", f32)
                A.activation(dummy2[:], thd[:, 0:1], Sin)

                # ======== phase 2 (fp16): dual-width scalar chain ========
                # N chain occupies columns [0,F) (scale qr), T chain [F,2F)
                # (scale s).  The noise translation's 0.6 = (0.03/0.05)
                # factors are applied at consumption (STT fusions below).
                thu = T(F2, "thu")
                V.tensor_mul(thu[:], SD, thd[:])
                sh = T(F2, "sh")
                A.activation(sh[:], thu[:], Sin, scale=0.5)
                ch = T(F2, "ch")                           # = [qNw | qTw]
                A.activation(ch[:], thu[:], Sin, scale=-0.5, bias=pih[:])
                sn = T(F2, "sn")
                A.activation(sn[:], thu[:], Sin)
                rth = T(F2, "rth")
                V.tensor_mul(rth[:], thd[:], rh2d[:])
                dd = T(F2, "dd")
                V.tensor_sub(dd[:], thu[:], sn[:])
                c1ad = T(F2, "c1ad")       # (thu-sin thu)/th = cc*th2
                V.tensor_mul(c1ad[:], dd[:], rth[:])
                ccd = T(F2, "ccd")         # (thu-sin thu)/th^3
                V.tensor_mul(ccd[:], c1ad[:], rh2d[:])
                qsd = T(F2, "qsd")
                V.tensor_mul(qsd[:], sh[:], rth[:])
                bbd = T(F2, "bbd")         # (1-cos thu)/th^2 = 2*qs^2
                A.activation(bbd[:], qsd[:], Square, scale=SQ2)

                def half(t, h):
                    return t[:, h * F:(h + 1) * F]

                def mk_quat(pre, w16, qs_ap):
                    qxyz = T(3 * F, pre + "qxyz")
                    w3 = AP(w16[:].tensor, w16[:].offset,
                            [list(w16[:].ap[0]), [F, 3], [1, F]])
                    qv = AP(qs_ap.tensor, qs_ap.offset,
                            [list(qs_ap.ap[0]), [0, 3], list(qs_ap.ap[-1])])
                    V.tensor_mul(c3(qxyz), qv, w3)
                    return qxyz

                dN = dict(qw=half(ch, 0), bb=half(bbd, 0), cc=half(ccd, 0),
                          c1a=half(c1ad, 0), qxyz=mk_quat("N", ns6, half(qsd, 0)))
                dT = dict(qw=half(ch, 1), bb=half(bbd, 1), cc=half(ccd, 1),
                          c1a=half(c1ad, 1), qxyz=mk_quat("T", tw6, half(qsd, 1)))

                # ======== crosses + translations (fp16) ========
                def cross(pre, a_t, aoff, b_t, boff, eng=None):
                    eng = eng or V
                    out = T(3 * F, pre)
                    for i in range(3):
                        j, k = (i + 1) % 3, (i + 2) % 3
                        m1 = pool.tile([P, F], f16, tag=pre + "m",
                                       name=pre + f"m{i}", bufs=3)
                        eng.tensor_mul(m1[:], pl(a_t, aoff + j), pl(b_t, boff + k))
                        m2 = pool.tile([P, F], f16, tag=pre + "n",
                                       name=pre + f"n{i}", bufs=3)
                        eng.tensor_mul(m2[:], pl(a_t, aoff + k), pl(b_t, boff + j))
                        eng.tensor_sub(pl(out, i), m1[:], m2[:])
                    return out

                MUL = mybir.AluOpType.mult
                SUB = mybir.AluOpType.subtract

                def bcap(a):
                    """[P,F]-shaped AP -> broadcast [P,3,F]."""
                    return AP(a.tensor, a.offset,
                              [list(a.ap[0]), [0, 3], list(a.ap[-1])])

                def translation(pre, w16, d, scale_t, out_ap, c_fix=None,
                                cross_eng=None, p_eng=None):
                    """out = scale_t*v + f*bb*(w x v) + f*cc*(w x (w x v))
                    with f = c_fix or 1, via w x (w x v) = w*(w.v) - th2*v:
                    out = (scale_t - f*c1a)*v + f*bb*(w x v) + (f*cc*(w.v))*w
                    (cc*th2 = c1a, per-sample planes).  [P,3,F]"""
                    eng = cross_eng or V
                    cr1 = cross(pre + "c1", w16, 0, w16, 3, eng=cross_eng)
                    w3 = AP(w16[:].tensor, w16[:].offset,
                            [list(w16[:].ap[0]), [F, 3], [1, F]])
                    v3 = AP(w16[:].tensor, w16[:].offset + 3 * F,
                            [list(w16[:].ap[0]), [F, 3], [1, F]])
                    dw = T(3 * F, pre + "dw")          # w .* v per component
                    eng.tensor_mul(c3(dw), w3, v3)
                    d1 = T(F, pre + "d1")
                    eng.tensor_add(d1[:], pl(dw, 0), pl(dw, 1))
                    dot = T(F, pre + "dot")
                    eng.tensor_add(dot[:], d1[:], pl(dw, 2))
                    alpha = T(F, pre + "al")           # scale_t - f*c1a
                    gamma = T(F, pre + "ga")           # f*cc*(w.v)
                    if c_fix is None:
                        eng.tensor_sub(alpha[:], scale_t, d["c1a"])
                        eng.tensor_mul(gamma[:], d["cc"], dot[:])
                        bb = d["bb"]
                    else:
                        eng.scalar_tensor_tensor(alpha[:], d["c1a"], -c_fix,
                                                 scale_t, op0=MUL, op1=ADD)
                        eng.scalar_tensor_tensor(gamma[:], d["cc"], c_fix,
                                                 dot[:], op0=MUL, op1=MUL)
                        bbf = T(F, pre + "bbf")
                        A.activation(bbf[:], d["bb"], Copy, scale=c_fix)
                        bb = bbf[:]
                    pe = p_eng or V
                    p1 = T(3 * F, pre + "p1")
                    pe.tensor_mul(c3(p1), bcap(alpha[:]), v3)
                    p2 = T(3 * F, pre + "p2")
                    pe.tensor_mul(c3(p2), bcap(bb), c3(cr1))
                    p3 = T(3 * F, pre + "p3")
                    pe.tensor_mul(c3(p3), bcap(gamma[:]), w3)
                    s1 = T(3 * F, pre + "s1")
                    pe.tensor_add(s1[:], p1[:], p2[:])
                    pe.tensor_add(out_ap, c3(s1), c3(p3))

                # constant rows (0,0,0,1) — emitted here so they don't block
                # the chain-pre squares at the head of Pool's queue
                for ov in (o0v, o1v):
                    G.memset(ov[:, :, 12:15], 0.0)
                    G.memset(ov[:, :, 15], 1.0)

                translation("Nt", ns6, dN, QT16, tri(stN, 3, 4), c_fix=0.6)
                tt = T(3 * F, "tt")
                translation("Tt", tw6, dT, S16, c3(tt), cross_eng=G)

                # ======== R(q) into staging (fp16) ========
                def rot_from_quat(pre, qw, qxyz, st, f0=0, fw=F):
                    """R entries for sample-columns [f0, f0+fw)."""
                    def w(t, k):      # windowed plane k of a tile
                        return t[:, k * F + f0: k * F + f0 + fw]

                    def w3(t, k0, dk):   # windowed triple (planes k0+i*dk)
                        a = t[:, k0 * F + f0: k0 * F + f0 + fw]
                        return AP(a.tensor, a.offset,
                                  [list(a.ap[0]), [dk * F, 3], list(a.ap[-1])])

                    def wbc(plane_t, k=0):   # windowed broadcast scalar plane
                        a = plane_t[:, k * F + f0: k * F + f0 + fw]
                        return AP(a.tensor, a.offset,
                                  [list(a.ap[0]), [0, 3], list(a.ap[-1])])

                    q2 = T(3 * fw, pre + "q2")
                    q2t = lambda k: q2[:, k * fw:(k + 1) * fw]
                    q23 = AP(q2[:].tensor, q2[:].offset,
                             [list(q2[:].ap[0]), [fw, 3], [1, fw]])
                    V.tensor_add(q23, w3(qxyz, 0, 1), w3(qxyz, 0, 1))
                    pd = T(3 * fw, pre + "pd")      # 2qx^2, 2qy^2, 2qz^2
                    pdt = lambda k: pd[:, k * fw:(k + 1) * fw]
                    pd3 = AP(pd[:].tensor, pd[:].offset,
                             [list(pd[:].ap[0]), [fw, 3], [1, fw]])
                    A.activation(pd3, w3(qxyz, 0, 1), Square, scale=SQ2)
                    pw = T(3 * fw, pre + "pw")      # 2 qw (qx,qy,qz)
                    pwt = lambda k: pw[:, k * fw:(k + 1) * fw]
                    pw3 = AP(pw[:].tensor, pw[:].offset,
                             [list(pw[:].ap[0]), [fw, 3], [1, fw]])
                    V.tensor_mul(pw3, wbc(qw), q23)
                    pxy = T(fw, pre + "pxy")
                    V.tensor_mul(pxy[:], q2t(0), w(qxyz, 1))
                    pxz = T(fw, pre + "pxz")
                    V.tensor_mul(pxz[:], q2t(0), w(qxyz, 2))
                    pyz = T(fw, pre + "pyz")
                    V.tensor_mul(pyz[:], q2t(1), w(qxyz, 2))
                    ds = T(3 * fw, pre + "ds")      # R_ii = 1 - (pd_j + pd_k)
                    dst = lambda k: ds[:, k * fw:(k + 1) * fw]
                    V.tensor_add(dst(0), pdt(1), pdt(2))
                    V.tensor_add(dst(1), pdt(0), pdt(2))
                    V.tensor_add(dst(2), pdt(0), pdt(1))
                    ds3 = AP(ds[:].tensor, ds[:].offset,
                             [list(ds[:].ap[0]), [fw, 3], [1, fw]])
                    A.activation(w3(st, 0, 5), ds3, Copy, scale=-1.0, bias=1.0)
                    V.tensor_sub(w(st, 1), pxy[:], pwt(2))
                    V.tensor_add(w(st, 4), pxy[:], pwt(2))
                    V.tensor_add(w(st, 2), pxz[:], pwt(1))
                    V.tensor_sub(w(st, 8), pxz[:], pwt(1))
                    V.tensor_sub(w(st, 6), pyz[:], pwt(0))
                    V.tensor_add(w(st, 9), pyz[:], pwt(0))

                rot_from_quat("Nr", dN["qw"], dN["qxyz"], stN)

                # ======== compose: qo = qN (x) qT (fp16) ========
                qNx, qTx = dN["qxyz"], dT["qxyz"]
                qNw, qTw = dN["qw"], dT["qw"]
                m0 = T(F, "m0"); V.tensor_mul(m0[:], qNw, qTw)
                md = T(3 * F, "md"); V.tensor_mul(md[:], qNx[:], qTx[:])
                md1 = T(F, "md1"); V.tensor_add(md1[:], pl(md, 0), pl(md, 1))
                md2 = T(F, "md2"); V.tensor_add(md2[:], md1[:], pl(md, 2))
                qow = T(F, "qow"); V.tensor_sub(qow[:], m0[:], md2[:])
                aN = T(3 * F, "aN")
                V.tensor_mul(c3(aN), bcap(qNw), c3(qTx))
                bN = T(3 * F, "bN")
                V.tensor_mul(c3(bN), bcap(qTw), c3(qNx))
                abN = T(3 * F, "abN"); V.tensor_add(abN[:], aN[:], bN[:])
                qcr = cross("qc", qNx, 0, qTx, 0)
                qoxyz = T(3 * F, "qoxyz"); V.tensor_add(qoxyz[:], abN[:], qcr[:])

                # ======== scatter staging -> f32 interleaved outputs ========
                def scatter(st, ov, eng, f0=0, fw=F):
                    a = st[:, f0:f0 + fw]
                    src = AP(a.tensor, a.offset,
                             [list(a.ap[0]), [1, fw], [F, 12]])
                    if eng is A:
                        eng.copy(ov[:, f0:f0 + fw, 0:12], src)
                    else:
                        eng.tensor_copy(ov[:, f0:f0 + fw, 0:12], src)

                scatter(stN, o1v, G)   # mid-kernel, overlaps compose
                nc.sync.dma_start(o1_d[:], o1[:])

                # ---- final stage in column-halves: R(qo), t_o, scatter, store
                # so the first half's DMA overlaps the second half's compute.
                def wtri(t, p0, dp, f0, fw):
                    a = t[:, p0 * F + f0: p0 * F + f0 + fw]
                    return AP(a.tensor, a.offset,
                              [list(a.ap[0]), [dp * F, 3], list(a.ap[-1])])

                H = F // 2
                for hi, f0 in enumerate(range(0, F, H)):
                    rot_from_quat(f"Or{hi}", qow, qoxyz, stO, f0=f0, fw=H)
                    # t_o = R_n @ tt + tn (windowed)
                    mm = T(9 * H, f"mm{hi}")
                    mmw = AP(mm[:].tensor, mm[:].offset,
                             [list(mm[:].ap[0]), [3 * H, 3], [H, 3], [1, H]])
                    a = stN[:, f0:f0 + H]
                    rn = AP(a.tensor, a.offset,
                            [list(a.ap[0]), [4 * F, 3], [F, 3], [1, H]])
                    a = tt[:, f0:f0 + H]
                    ttb = AP(a.tensor, a.offset,
                             [list(a.ap[0]), [0, 3], [F, 3], [1, H]])
                    V.tensor_mul(mmw, rn, ttb)
                    ms1 = T(3 * H, f"ms1{hi}")
                    ms13 = AP(ms1[:].tensor, ms1[:].offset,
                              [list(ms1[:].ap[0]), [H, 3], [1, H]])
                    V.tensor_add(ms13,
                                 AP(mm[:].tensor, mm[:].offset,
                                    [list(mm[:].ap[0]), [3 * H, 3], [1, H]]),
                                 AP(mm[:].tensor, mm[:].offset + H,
                                    [list(mm[:].ap[0]), [3 * H, 3], [1, H]]))
                    ms2 = T(3 * H, f"ms2{hi}")
                    ms23 = AP(ms2[:].tensor, ms2[:].offset,
                              [list(ms2[:].ap[0]), [H, 3], [1, H]])
                    V.tensor_add(ms23, ms13,
                                 AP(mm[:].tensor, mm[:].offset + 2 * H,
                                    [list(mm[:].ap[0]), [3 * H, 3], [1, H]]))
                    V.tensor_add(wtri(stO, 3, 4, f0, H), ms23,
                                 wtri(stN, 3, 4, f0, H))
                    # h1 scatter on ACT (overlaps h2 compute on DVE); h2 on
                    # the by-then-idle DVE, whose copy is 2x faster -> tail
                    scatter(stO, o0v, A if hi == 0 else V, f0=f0, fw=H)
                    nc.sync.dma_start(o0_d[:, f0 * 16:(f0 + H) * 16],
                                      o0[:, f0 * 16:(f0 + H) * 16])

    nc.compile()
    return nc


def _make_runner(nc):
    """Compile a Bass program into a cached 8-core jitted callable."""
    import jax
    from jax.sharding import Mesh, PartitionSpec
    from jax.experimental.shard_map import shard_map
    import concourse.mybir as mybir
    from concourse import bass2jax

    bass2jax.install_neuronx_cc_hook()

    in_names, out_names, out_avals = [], [], []
    partition_name = nc.partition_id_tensor.name if nc.partition_id_tensor else None
    for alloc in nc.m.functions[0].allocations:
        if not isinstance(alloc, mybir.MemoryLocationSet):
            continue
        name = alloc.memorylocations[0].name
        if alloc.kind == "ExternalInput":
            if name != partition_name:
                in_names.append(name)
        elif alloc.kind == "ExternalOutput":
            out_names.append(name)
            out_avals.append(jax.core.ShapedArray(
                tuple(alloc.tensor_shape), mybir.dt.np(alloc.dtype)))
    n_params = len(in_names)
    all_names = in_names + out_names + ([partition_name] if partition_name else [])

    def _body(*args):
        operands = list(args)
        if partition_name is not None:
            operands.append(bass2jax.partition_id_tensor())
        outs = bass2jax._bass_exec_p.bind(
            *operands,
            out_avals=tuple(out_avals),
            in_names=tuple(all_names),
            out_names=tuple(out_names),
            lowering_input_output_aliases=(),
            sim_require_finite=True,
            sim_require_nnan=True,
            nc=nc,
        )
        return tuple(outs)

    devices = jax.devices()[:N_CORES]
    mesh = Mesh(np.asarray(devices), ("core",))
    n_outs = len(out_avals)
    sharded = jax.jit(shard_map(
        _body, mesh=mesh,
        in_specs=(PartitionSpec("core"),) * (n_params + n_outs),
        out_specs=(PartitionSpec("core"),) * n_outs,
        check_rep=False), keep_unused=True)

    zeros = [np.zeros((N_CORES * a.shape[0],) + tuple(a.shape[1:]), a.dtype)
             for a in out_avals]

    def run(concat_inputs):
        args = [concat_inputs[n] for n in in_names] + zeros
        outs = sharded(*args)
        return {n: np.asarray(o) for n, o in zip(out_names, outs)}

    return run, in_names, out_names, sharded, zeros, mesh


def _get_runner():
    if "runner" not in _CACHE:
        run, in_names, out_names, sharded, zeros, mesh = _make_runner(_build_program())
        _CACHE["runner"] = (run, in_names, out_names)
        _CACHE["sharded"] = (sharded, in_names, out_names, zeros, mesh)
    return _CACHE["runner"]


def _host_prep(twist, noise, alpha_bars, timesteps):
    f = np.float32
    h = np.float16
    ab = np.asarray(alpha_bars, f)[np.asarray(timesteps)]          # (B,)
    s = np.sqrt(ab).astype(h)
    q = np.sqrt((1.0 - ab).astype(f))
    qr = (np.float32(0.05) * q).astype(h)
    qt = (np.float32(0.03) * q).astype(h)

    def planes6(x):
        # (B,HO,6) -> (N_CORES*P, 6F): per core planes c-major, sample p*F+f
        x = np.asarray(x, f).astype(h).reshape(N_CORES, P, F, 6)
        return np.ascontiguousarray(x.transpose(0, 1, 3, 2)).reshape(N_CORES * P, 6 * F)

    def planes_scalar(*vs):
        cols = [np.broadcast_to(v.reshape(N_CORES, BL, 1), (N_CORES, BL, HO))
                .reshape(N_CORES, P, 1, F) for v in vs]
        return np.ascontiguousarray(
            np.concatenate(cols, axis=2)).reshape(N_CORES * P, len(vs) * F)

    return {"tw": planes6(twist), "ns": planes6(noise),
            "sq": planes_scalar(qr, s, qt)}   # [qr|s] dual scale + qt


def _unpack(out_concat):
    # (N_CORES*P, 16F) interleaved -> (B, HO, 4, 4)
    return out_concat.reshape(N_CORES, P * F, 16).reshape(B, HO, 4, 4)


def kernel(twist, noise, alpha_bars, timesteps):
    run, in_names, out_names = _get_runner()
    ins = _host_prep(twist, noise, alpha_bars, timesteps)
    for _attempt in range(3):
        outs = run(ins)
        # guard against rare transient NaNs seen once over the axon path
        if not any(np.isnan(v).any() for v in outs.values()):
            break
    return _unpack(outs["o0"]), _unpack(outs["o1"])


if __name__ == "__main__":
    rng = np.random.default_rng(0)
    tw = 0.5 * rng.standard_normal((B, HO, 6), dtype=np.float32)
    ns = rng.standard_normal((B, HO, 6), dtype=np.float32)
    ab = np.linspace(0.999, 1e-4, 100, dtype=np.float32)
    ts = rng.integers(0, 100, size=(B,)).astype(np.int32)
    o0, o1 = kernel(tw, ns, ab, ts)
    print("ok", o0.shape, o1.shape, o0.dtype)



# revision 2
# speedup vs baseline: 1.2076x; 1.2076x over previous
"""SE(3) diffusion scheduler add-noise kernel for 8 Trainium2 NeuronCores.

Math: reference computes
    orig = se3_exp(twist); xi = se3_log(inv(orig));
    H_t = se3_exp((1-sqrt(ab))*xi) @ orig;  H_n = se3_exp(sqrt(1-ab)*scale*noise)
    out0 = H_n @ H_t; out1 = H_n
Since exp(a*xi)exp(b*xi) = exp((a+b)*xi) on the one-parameter subgroup and
rotation angles stay < pi here (twist = 0.5*randn), xi = -twist exactly and
    H_t = se3_exp(sqrt(ab) * twist).

Host folds the per-sample scalars into the inputs during the fp16 layout
pass: w' = sqrt(ab)*twist_rot, v'' = sqrt(ab)*twist_trans,
n' = 0.05*sqrt(1-ab)*noise_rot, m' = 0.03*sqrt(1-ab)*noise_trans, each sent
as 5 planes [x y z x y] (cyclic extension makes cross products affine APs).

Device (per core, 32768 samples as [128 part x 256 free] planes, fp16):
  T chain: u = |w'|^2, th = sqrt(u), 1/u via f32 fast-reciprocal;
    quaternion (cos(th/2), sin(th/2)/th * w'); A = sin th/th, B = (1-cos)/u,
    C = (1-A)/u; t_T = A v'' + B (w' x v'') + C (w'.v'') w'.
  N chain: theta <= ~0.3 so every coefficient is affine in u_N
    (error < 1e-4): qw = 1-u/8, sig = 0.5-u/48, alpha = 1-u/6, B = 0.5-u/24;
    the C*(n.m)n term (<=1.5e-2 of the tiny t_N) is dropped.  No trig/sqrt.
  N quaternion carries a sqrt(2) factor (free: folded into the affine
  consts) so R(q) needs no doubling: with q' = sqrt2*q, R entries are plain
  products q'_i q'_j, diag = 1 - (pd_j+pd_k), pd = q'^2.  Compose
  q_O' = q_N' (x) q_T keeps the sqrt2 scale.  t_O = R_N t_T + t_N.

Outputs go to DRAM as the fp16 12-plane staging itself (planes = row-major
[R|t] entries); the host upcasts to f32 and appends the constant (0,0,0,1)
row.  Engine split: DVE bulk TT at 2x + affine TSP at 4x, ACT all table
funcs/squares/diagonals (one table switch: sqrt set -> trig set), Pool the
R(q_N) products + off-diagonals.
"""

import os
import sys

import numpy as np

for _p in ("/opt/trn_rl_repo", "/root/.axon_site/_ro/trn_rl_repo"):
    if os.path.isdir(_p) and _p not in sys.path:
        sys.path.append(_p)

N_CORES = 8
B, HO = 4096, 64
BL = B // N_CORES           # 512 rows per core
NS = BL * HO                # 32768 samples per core
P, F = 128, 256             # plane geometry: NS = P*F
PI_HALF = 1.5707963267948966
SQ2 = 1.4142135623730951
UEPS = 3.2e-5               # keeps 2/u below fp16 max after f32 reciprocal

_CACHE: dict = {}

# input plane offsets (each group 5 planes: x y z x y)
W, N, VV, M = 0, 5, 10, 15


def _build_program():
    import concourse.bacc as bacc
    import concourse.mybir as mybir
    import concourse.tile as tile
    from concourse.bass import AP

    f32 = mybir.dt.float32
    f16 = mybir.dt.float16
    Sin = mybir.ActivationFunctionType.Sin
    Sqrt = mybir.ActivationFunctionType.Sqrt
    Square = mybir.ActivationFunctionType.Square
    Copy = mybir.ActivationFunctionType.Copy
    MUL = mybir.AluOpType.mult
    ADD = mybir.AluOpType.add

    nc = bacc.Bacc("TRN2", target_bir_lowering=False, debug=False, num_devices=1)

    xi_d = nc.dram_tensor("xi", [P, 20 * F], f16, kind="ExternalInput").ap()
    o0_d = nc.dram_tensor("o0", [P, 12 * F], f16, kind="ExternalOutput").ap()
    o1_d = nc.dram_tensor("o1", [P, 12 * F], f16, kind="ExternalOutput").ap()

    def mk(t, plane, dims):
        """AP into tile t at plane offset, dims = [[stride_cols, n], ...]
        (innermost [1, F] appended automatically)."""
        a = t[:]
        return AP(a.tensor, a.offset + plane * F,
                  [list(a.ap[0])] + [[d[0] * F, d[1]] for d in dims] + [[1, F]])

    def pl(t, k, n=1):
        return t[:, k * F:(k + n) * F]

    n_reps = int(os.environ.get("KERNEL_REPS", "1"))

    with tile.TileContext(nc) as tc:
        with tc.tile_pool(name="w", bufs=1) as pool:
            V, A, G = nc.vector, nc.scalar, nc.gpsimd

            def T(cols, tag, dt=f16):
                return pool.tile([P, cols], dt, tag=tag, name=tag)

            for _rep in range(n_reps):
                xi = T(20 * F, "xi")
                nc.sync.dma_start(xi[:, 0:10 * F], xi_d[:, 0:10 * F])
                nc.sync.dma_start(xi[:, 10 * F:20 * F], xi_d[:, 10 * F:20 * F])

                st = T(24 * F, "st")          # planes 0-11: out1, 12-23: out0

                pih = T(1, "pih", f32)
                G.memset(pih[:], PI_HALF)
                # preload sqrt act-table set while the input DMAs run
                dummy = T(1, "dummy", f32)
                A.activation(dummy[:], pih[:], Sqrt)

                # ---- u = |w'|^2 (T), |n'|^2 (N) --------------------------
                sq = T(6 * F, "sq")
                A.activation(mk(sq, 0, [[3, 2], [1, 3]]),
                             mk(xi, 0, [[5, 2], [1, 3]]), Square)
                t1 = T(2 * F, "t1")
                V.tensor_add(t1[:], mk(sq, 0, [[3, 2]]), mk(sq, 1, [[3, 2]]))
                ud = T(2 * F, "ud")           # [u_T | u_N]
                V.tensor_add(ud[:], t1[:], mk(sq, 2, [[3, 2]]))

                ue = T(F, "ue", f32)
                V.tensor_scalar(ue[:], pl(ud, 0), UEPS, None, op0=ADD)
                rh2f = T(F, "rh2f", f32)
                V.reciprocal_approx_fast(rh2f[:], ue[:])
                rh2 = T(F, "rh2")             # = 2/u_T in fp16
                V.tensor_scalar(rh2[:], rh2f[:], 2.0, None, op0=MUL)

                th = T(F, "th")
                A.activation(th[:], pl(ud, 0), Sqrt)
                # switch to the trig table set now (load hides under DVE work)
                dummy2 = T(1, "dummy2", f32)
                A.activation(dummy2[:], th[:, 0:1], Sin)

                # ---- N-chain affine coefficients (DVE tensor_scalar, 4x) --
                wc = T(2 * F, "wc")           # [ch | qwN'] adjacency for ba
                uN = pl(ud, 1)
                V.tensor_scalar(pl(wc, 1), uN, -SQ2 / 8.0, SQ2,
                                op0=MUL, op1=ADD)            # qwN' = sq2(1-u/8)
                sgN = T(F, "sgN")
                V.tensor_scalar(sgN[:], uN, -SQ2 / 48.0, SQ2 / 2.0,
                                op0=MUL, op1=ADD)            # sq2(0.5-u/48)
                cf = T(5 * F, "cf")           # [A | alphaN | B_T | B_N | C_T]
                V.tensor_scalar(pl(cf, 1), uN, -1.0 / 6.0, 1.0,
                                op0=MUL, op1=ADD)
                V.tensor_scalar(pl(cf, 3), uN, -1.0 / 24.0, 0.5,
                                op0=MUL, op1=ADD)

                # ---- T-chain coefficients --------------------------------
                sh = T(F, "sh")
                A.activation(sh[:], th[:], Sin, scale=0.5)
                A.activation(pl(wc, 0), th[:], Sin, scale=-0.5, bias=pih[:])

                rt2 = T(F, "rt2")             # 2/th
                V.tensor_mul(rt2[:], th[:], rh2[:])
                sp = T(F, "sp")               # 2 sin(th/2)/th
                V.tensor_mul(sp[:], sh[:], rt2[:])
                sT = T(F, "sT")               # sin(th/2)/th
                V.tensor_scalar(sT[:], sp[:], 0.5, None, op0=MUL)
                V.tensor_mul(pl(cf, 0), sp[:], pl(wc, 0))    # A = sin th/th
                V.tensor_mul(pl(cf, 2), sT[:], sp[:])        # B = 2 sT^2
                d2 = T(F, "d2")
                V.tensor_scalar(d2[:], pl(cf, 0), -0.5, 0.5,
                                op0=MUL, op1=ADD)            # (1-A)/2
                V.tensor_mul(pl(cf, 4), d2[:], rh2[:])       # C = (1-A)/u

                # ---- quaternion vectors (extended 5-plane) ---------------
                q10 = T(10 * F, "q10")        # [qvN' e5 | qvT e5]
                V.tensor_mul(mk(q10, 0, [[1, 5]]), mk(sgN, 0, [[0, 5]]),
                             mk(xi, N, [[1, 5]]))
                V.tensor_mul(mk(q10, 5, [[1, 5]]), mk(sT, 0, [[0, 5]]),
                             mk(xi, W, [[1, 5]]))

                # ---- crosses w' x v'' and n' x m' (fused T+N) ------------
                cm1 = T(6 * F, "cm1")
                V.tensor_mul(mk(cm1, 0, [[3, 2], [1, 3]]),
                             mk(xi, 1, [[5, 2], [1, 3]]),
                             mk(xi, VV + 2, [[5, 2], [1, 3]]))
                cm2 = T(6 * F, "cm2")
                V.tensor_mul(mk(cm2, 0, [[3, 2], [1, 3]]),
                             mk(xi, 2, [[5, 2], [1, 3]]),
                             mk(xi, VV + 1, [[5, 2], [1, 3]]))
                cr = T(6 * F, "cr")           # [crT | crN]
                V.tensor_sub(cr[:], cm1[:], cm2[:])

                # ---- R(q_N') on Pool + ACT -------------------------------
                pdN = T(5 * F, "pdN")
                A.activation(pdN[:], pl(q10, 0, 5), Square)
                pwN = T(5 * F, "pwN")
                G.tensor_mul(mk(pwN, 0, [[1, 5]]), mk(wc, 1, [[0, 5]]),
                             mk(q10, 0, [[1, 5]]))
                offN = T(3 * F, "offN")
                G.tensor_mul(offN[:], pl(q10, 0, 3), pl(q10, 1, 3))
                dsN = T(3 * F, "dsN")
                G.tensor_add(dsN[:], pl(pdN, 1, 3), pl(pdN, 2, 3))
                A.activation(mk(st, 0, [[5, 3]]), mk(dsN, 0, [[1, 3]]),
                             Copy, scale=-1.0, bias=1.0)
                G.tensor_sub(pl(st, 1), pl(offN, 0), pl(pwN, 2))
                G.tensor_add(pl(st, 4), pl(offN, 0), pl(pwN, 2))
                G.tensor_add(pl(st, 2), pl(offN, 2), pl(pwN, 1))
                G.tensor_sub(pl(st, 8), pl(offN, 2), pl(pwN, 1))
                G.tensor_sub(pl(st, 6), pl(offN, 1), pl(pwN, 0))
                G.tensor_add(pl(st, 9), pl(offN, 1), pl(pwN, 0))

                # ---- dot (T only) + translations -------------------------
                pr3 = T(3 * F, "pr3")
                V.tensor_mul(pr3[:], pl(xi, W, 3), pl(xi, VV, 3))
                dt1 = T(F, "dt1")
                V.tensor_add(dt1[:], pl(pr3, 0), pl(pr3, 1))
                dot = T(F, "dot")
                V.tensor_add(dot[:], dt1[:], pl(pr3, 2))
                ga = T(F, "ga")
                V.tensor_mul(ga[:], pl(cf, 4), dot[:])       # C*(w'.v'')

                tp = T(6 * F, "tp")
                V.tensor_mul(mk(tp, 0, [[3, 2], [1, 3]]),
                             mk(cf, 0, [[1, 2], [0, 3]]),
                             mk(xi, VV, [[5, 2], [1, 3]]))
                tq = T(6 * F, "tq")
                V.tensor_mul(mk(tq, 0, [[3, 2], [1, 3]]),
                             mk(cf, 2, [[1, 2], [0, 3]]),
                             mk(cr, 0, [[3, 2], [1, 3]]))
                tr = T(3 * F, "tr")
                V.tensor_mul(mk(tr, 0, [[1, 3]]), mk(ga, 0, [[0, 3]]),
                             mk(xi, W, [[1, 3]]))
                ts = T(6 * F, "ts")
                V.tensor_add(ts[:], tp[:], tq[:])
                tt = T(3 * F, "tt")           # t_T
                V.tensor_add(tt[:], pl(ts, 0, 3), tr[:])
                V.tensor_copy(mk(st, 3, [[4, 3]]), mk(ts, 3, [[1, 3]]))

                nc.sync.dma_start(o1_d[:], st[:, 0:12 * F])

                # ---- compose q_O' = q_N' (x) q_T -------------------------
                m0 = T(F, "m0")
                V.tensor_mul(m0[:], pl(wc, 0), pl(wc, 1))
                md = T(3 * F, "md")
                V.tensor_mul(md[:], pl(q10, 0, 3), pl(q10, 5, 3))
                md1 = T(F, "md1")
                V.tensor_add(md1[:], pl(md, 0), pl(md, 1))
                md2 = T(F, "md2")
                V.tensor_add(md2[:], md1[:], pl(md, 2))
                qow = T(F, "qow")
                V.tensor_sub(qow[:], m0[:], md2[:])
                ba = T(6 * F, "ba")           # [qwT*qvN' | qwN'*qvT]
                V.tensor_mul(mk(ba, 0, [[3, 2], [1, 3]]),
                             mk(wc, 0, [[1, 2], [0, 3]]),
                             mk(q10, 0, [[5, 2], [1, 3]]))
                ab = T(3 * F, "ab")
                V.tensor_add(ab[:], pl(ba, 0, 3), pl(ba, 3, 3))
                qm1 = T(3 * F, "qm1")
                V.tensor_mul(qm1[:], pl(q10, 1, 3), pl(q10, 7, 3))
                qm2 = T(3 * F, "qm2")
                V.tensor_mul(qm2[:], pl(q10, 2, 3), pl(q10, 6, 3))
                qcr = T(3 * F, "qcr")
                V.tensor_sub(qcr[:], qm1[:], qm2[:])
                qo = T(5 * F, "qo")
                V.tensor_add(pl(qo, 0, 3), ab[:], qcr[:])
                V.tensor_copy(pl(qo, 3, 2), pl(qo, 0, 2))

                # ---- R(q_O') ---------------------------------------------
                pdO = T(5 * F, "pdO")
                A.activation(pdO[:], qo[:], Square)
                pwO = T(5 * F, "pwO")
                V.tensor_mul(mk(pwO, 0, [[1, 5]]), mk(qow, 0, [[0, 5]]),
                             mk(qo, 0, [[1, 5]]))
                offO = T(3 * F, "offO")
                V.tensor_mul(offO[:], pl(qo, 0, 3), pl(qo, 1, 3))
                dsO = T(3 * F, "dsO")
                V.tensor_add(dsO[:], pl(pdO, 1, 3), pl(pdO, 2, 3))
                A.activation(mk(st, 12, [[5, 3]]), mk(dsO, 0, [[1, 3]]),
                             Copy, scale=-1.0, bias=1.0)
                V.tensor_sub(pl(st, 13), pl(offO, 0), pl(pwO, 2))
                V.tensor_add(pl(st, 16), pl(offO, 0), pl(pwO, 2))
                V.tensor_add(pl(st, 14), pl(offO, 2), pl(pwO, 1))
                V.tensor_sub(pl(st, 20), pl(offO, 2), pl(pwO, 1))
                V.tensor_sub(pl(st, 18), pl(offO, 1), pl(pwO, 0))
                V.tensor_add(pl(st, 21), pl(offO, 1), pl(pwO, 0))

                # ---- t_O = R_N t_T + t_N ---------------------------------
                mm = T(9 * F, "mm")
                V.tensor_mul(mk(mm, 0, [[3, 3], [1, 3]]),
                             mk(st, 0, [[4, 3], [1, 3]]),
                             mk(tt, 0, [[0, 3], [1, 3]]))
                s1 = T(3 * F, "s1")
                V.tensor_add(s1[:], mk(mm, 0, [[3, 3]]), mk(mm, 1, [[3, 3]]))
                s2 = T(3 * F, "s2")
                V.tensor_add(s2[:], s1[:], mk(mm, 2, [[3, 3]]))
                V.tensor_add(mk(st, 15, [[4, 3]]), mk(s2, 0, [[1, 3]]),
                             mk(st, 3, [[4, 3]]))

                nc.sync.dma_start(o0_d[:], st[:, 12 * F:24 * F])

    nc.compile()
    return nc


def _make_runner(nc):
    """Compile a Bass program into a cached 8-core jitted callable."""
    import jax
    from jax.sharding import Mesh, PartitionSpec
    from jax.experimental.shard_map import shard_map
    import concourse.mybir as mybir
    from concourse import bass2jax

    bass2jax.install_neuronx_cc_hook()

    in_names, out_names, out_avals = [], [], []
    partition_name = nc.partition_id_tensor.name if nc.partition_id_tensor else None
    for alloc in nc.m.functions[0].allocations:
        if not isinstance(alloc, mybir.MemoryLocationSet):
            continue
        name = alloc.memorylocations[0].name
        if alloc.kind == "ExternalInput":
            if name != partition_name:
                in_names.append(name)
        elif alloc.kind == "ExternalOutput":
            out_names.append(name)
            out_avals.append(jax.core.ShapedArray(
                tuple(alloc.tensor_shape), mybir.dt.np(alloc.dtype)))
    n_params = len(in_names)
    all_names = in_names + out_names + ([partition_name] if partition_name else [])

    def _body(*args):
        operands = list(args)
        if partition_name is not None:
            operands.append(bass2jax.partition_id_tensor())
        outs = bass2jax._bass_exec_p.bind(
            *operands,
            out_avals=tuple(out_avals),
            in_names=tuple(all_names),
            out_names=tuple(out_names),
            lowering_input_output_aliases=(),
            sim_require_finite=True,
            sim_require_nnan=True,
            nc=nc,
        )
        return tuple(outs)

    devices = jax.devices()[:N_CORES]
    mesh = Mesh(np.asarray(devices), ("core",))
    n_outs = len(out_avals)
    sharded = jax.jit(shard_map(
        _body, mesh=mesh,
        in_specs=(PartitionSpec("core"),) * (n_params + n_outs),
        out_specs=(PartitionSpec("core"),) * n_outs,
        check_rep=False), keep_unused=True)

    zeros = [np.zeros((N_CORES * a.shape[0],) + tuple(a.shape[1:]), a.dtype)
             for a in out_avals]

    def run(concat_inputs):
        args = [concat_inputs[n] for n in in_names] + zeros
        outs = sharded(*args)
        return {n: np.asarray(o) for n, o in zip(out_names, outs)}

    return run, in_names, out_names, sharded, zeros, mesh


def _get_runner():
    if "runner" not in _CACHE:
        run, in_names, out_names, sharded, zeros, mesh = _make_runner(_build_program())
        _CACHE["runner"] = (run, in_names, out_names)
        _CACHE["sharded"] = (sharded, in_names, out_names, zeros, mesh)
    return _CACHE["runner"]


def _host_prep(twist, noise, alpha_bars, timesteps):
    f, h = np.float32, np.float16
    ab = np.asarray(alpha_bars, f)[np.asarray(timesteps)]          # (B,)
    s = np.sqrt(ab)[:, None, None]
    q = np.sqrt(1.0 - ab)[:, None, None]
    tw = np.asarray(twist, f)
    ns = np.asarray(noise, f)

    def gext(x):
        # (B,HO,3) f32 -> (8,P,5,F) fp16, planes [x y z x y]
        x = x.astype(h).reshape(N_CORES, P, F, 3).transpose(0, 1, 3, 2)
        return np.concatenate([x, x[:, :, 0:2]], axis=2)

    xi = np.concatenate([gext(tw[..., 0:3] * s), gext(ns[..., 0:3] * (0.05 * q)),
                         gext(tw[..., 3:6] * s), gext(ns[..., 3:6] * (0.03 * q))],
                        axis=2)
    return {"xi": np.ascontiguousarray(xi).reshape(N_CORES * P, 20 * F)}


_BOTTOM = np.array([0.0, 0.0, 0.0, 1.0], np.float32)


def _unpack(o):
    # (8P, 12F) fp16 planes -> (B, HO, 4, 4) f32 with constant bottom row
    x = o.reshape(N_CORES, P, 12, F).transpose(0, 1, 3, 2)
    out = np.empty((B, HO, 4, 4), np.float32)
    out[..., :3, :] = x.reshape(B, HO, 3, 4)
    out[..., 3, :] = _BOTTOM
    return out


def kernel(twist, noise, alpha_bars, timesteps):
    run, in_names, out_names = _get_runner()
    ins = _host_prep(twist, noise, alpha_bars, timesteps)
    for _attempt in range(3):
        outs = run(ins)
        # guard against rare transient NaNs seen once over the axon path
        if not any(np.isnan(v).any() for v in outs.values()):
            break
    return _unpack(outs["o0"]), _unpack(outs["o1"])


if __name__ == "__main__":
    rng = np.random.default_rng(0)
    tw = 0.5 * rng.standard_normal((B, HO, 6), dtype=np.float32)
    ns = rng.standard_normal((B, HO, 6), dtype=np.float32)
    ab = np.linspace(0.999, 1e-4, 100, dtype=np.float32)
    ts = rng.integers(0, 100, size=(B,)).astype(np.int32)
    o0, o1 = kernel(tw, ns, ab, ts)
    print("ok", o0.shape, o1.shape, o0.dtype)
